# revision 41
# baseline (speedup 1.0000x reference)
"""Causal self-attention (B=4, T=2048, C=1024, H=16, D=64) on 8 trn2 NeuronCores.

Sharding: core c = (batch b=c//2, head-group hg=c%2 of 8 heads / 512 channels).
Each core computes attention for its 8 heads on its batch plus the partial
output projection over its 512 channels of Wp; the host sums the two partial
projections per batch and adds bp.

Per-core layout is feature-major ("transposed"): x is sent as xT (C, T) so
q/k project directly as qT = Wq.T @ x.T with both operands k(partition)-major.
v is computed in natural (T, D) orientation with a ones-column appended per
head so that the yT = [v|1].T @ P^T matmul also yields softmax row sums.
Matmul operands are bf16 (1 cyc/row on the PE); accumulation, softmax
internals and the final output stay fp32.

All attention matmuls are geometrically FULL 128x128-array ops (kT stored
twice per pair with complementary zero halves; v blocks padded to 128 wide)
so the PE HAM activity monitor keeps the clock gate at K=8/8 (2.4 GHz)
instead of dropping to 4/8 on the 64-row/65-col attention shapes.

Schedule: phase V (v for all heads, DMA-paced by sequence-half x loads) ->
QK(pair 0) m-outer -> per pair p: attention (software-pipelined per head:
yT matmuls of key-tile j-1 interleave between the S-matmul sections of
key-tile j), with independent full-array PE work streamed one item per
S-section into the PE's exp-wait gaps: the QK projection of pair p+1 for
p<3, and the output projection (gated on incremental per-chunk softmax
normalization) for p=3 and the tail.  The ACT engine runs only the softmax
exp during attention; all steady-state evictions go through the DVE.
"""

import math
from collections import deque

import numpy as np

B, T, C = 4, 2048, 1024
H, D = 16, 64
NCORES = 8
PAIRS = 4          # head pairs per core (2 heads = 128 channels each)
KT = C // 128      # 8 k-tiles over input channels
MT = T // 128      # 16 tiles over sequence
SC = 1.0 / math.sqrt(D)

_CACHE = {}


def _build_nc():
    from contextlib import ExitStack

    import concourse.bacc as bacc
    import concourse.mybir as mybir
    import concourse.tile as tile

    f32 = mybir.dt.float32
    bf16 = mybir.dt.bfloat16
    AF = mybir.ActivationFunctionType

    nc = bacc.Bacc("TRN2", target_bir_lowering=False, debug=False)

    xT = nc.dram_tensor("xT", (C, T), bf16, kind="ExternalInput").ap()
    wqD = nc.dram_tensor("wq", (C, 512), bf16, kind="ExternalInput").ap()
    wkD = nc.dram_tensor("wk", (C, 512), bf16, kind="ExternalInput").ap()
    wvD = nc.dram_tensor("wv", (C, 512), bf16, kind="ExternalInput").ap()
    wpD = nc.dram_tensor("wp", (512, C), bf16, kind="ExternalInput").ap()
    bqD = nc.dram_tensor("bq", (512,), f32, kind="ExternalInput").ap()
    bkD = nc.dram_tensor("bk", (512,), f32, kind="ExternalInput").ap()
    bvD = nc.dram_tensor("bv", (512,), f32, kind="ExternalInput").ap()
    # partial projections leave the core in bf16: halves the 8MB writeback
    # (it is ring-bandwidth-bound in the drain tail); the host sums the two
    # per-batch partials in fp32.
    outD = nc.dram_tensor("out", (T, C), bf16, kind="ExternalOutput").ap()

    with tile.TileContext(nc) as tc, ExitStack() as ctx:
        const = ctx.enter_context(tc.tile_pool(name="const", bufs=1))
        xp = ctx.enter_context(tc.tile_pool(name="xp", bufs=1))

        wv_sb = const.tile([128, KT, 512], bf16)
        xsb = [xp.tile([128, T], bf16, name=f"xsb{k}") for k in range(KT)]
        wq_sb = const.tile([128, KT, 512], bf16)
        wk_sb = const.tile([128, KT, 512], bf16)
        wp_sb = const.tile([128, 4, C], bf16)
        wz = const.tile([128, 512], bf16)     # warm-up zeros
        wdum = const.tile([128, 8], bf16)     # dummy exp target

        # ---- warm-up: the HAM clock gate defaults to 4/8 (1.2 GHz) and
        # un-throttles only after ~3.4us of sustained PE activity; dummy
        # matmuls from t~0 cover the initial DMA wait so phase 0 runs at
        # 2.4 GHz.  The dummy exp pulls the ACT table-set load (~2.7us)
        # out of the first attention row.
        gpsum = tc.alloc_tile_pool(name="gpsum", bufs=4, space="PSUM")
        wps = tc.alloc_tile_pool(name="wps", bufs=1, space="PSUM")
        wt = wps.tile([128, 512], f32, name="warm")
        nc.vector.memset(wz[:], 0.0)
        nc.scalar.activation(wdum[:], wz[:, 0:8], AF.Exp, scale=1.0)
        for _ in range(16):
            nc.tensor.matmul(wt[:], lhsT=wz[:, 0:128], rhs=wz[:], start=True,
                             stop=True)

        def warm_mm(n=1):
            # dummy self-loading matmuls (~50ns each issue-to-issue when
            # overlapped) to pad PE activity across known DMA waits; never
            # use standalone ldweights (walrus pairs it with the next real
            # matmul, which then consumes the dummy weights).
            for _ in range(n):
                nc.tensor.matmul(wt[:, 0:128], lhsT=wz[:, 0:128],
                                 rhs=wz[:, 0:128], start=True, stop=True)

        # DMA issue order = first-needed first.  Phase 0 only consumes
        # x cols 0:1024 (V t0-3, q m0/m1) plus wv/wq/wk; everything else
        # streams in behind attention's first rows.  Alternate big
        # transfers across both HWDGE rings (SP + ACT).
        bq_sb = const.tile([128, PAIRS], f32)
        nc.sync.dma_start(bq_sb[:], bqD.rearrange("(a p) -> p a", p=128))
        bk_sb = const.tile([128, PAIRS], f32)
        nc.scalar.dma_start(bk_sb[:], bkD.rearrange("(a p) -> p a", p=128))
        bv_row = const.tile([1, 512], f32)
        nc.sync.dma_start(bv_row[:], bvD.rearrange("(a n) -> a n", a=1))
        bv_bc = const.tile([128, 512], f32)
        nc.gpsimd.partition_broadcast(bv_bc[:], bv_row[:])
        wv4 = wvD.rearrange("(k p) n -> p k n", p=128)
        wq4 = wqD.rearrange("(k p) n -> p k n", p=128)
        wk4 = wkD.rearrange("(k p) n -> p k n", p=128)
        wp4 = wpD.rearrange("(k p) n -> p k n", p=128)
        for k in range(KT):
            exs = nc.sync if k % 2 == 0 else nc.scalar
            ewv = nc.scalar if k % 2 == 0 else nc.sync
            ewv.dma_start(wv_sb[:, k, :], wv4[:, k, :])
            exs.dma_start(xsb[k][:, 0:512], xT[k * 128:(k + 1) * 128, 0:512])
        for k in range(KT):
            exs = nc.scalar if k % 2 == 0 else nc.sync
            exs.dma_start(xsb[k][:, 512:1024],
                          xT[k * 128:(k + 1) * 128, 512:1024])
        for k in range(KT):
            exs = nc.sync if k % 2 == 0 else nc.scalar
            exs.dma_start(
                xsb[k][:, 1024:2048], xT[k * 128:(k + 1) * 128, 1024:2048]
            )
        nc.scalar.dma_start(wq_sb[:, 0:4, :], wq4[:, 0:4, :])
        nc.sync.dma_start(wq_sb[:, 4:8, :], wq4[:, 4:8, :])
        nc.scalar.dma_start(wk_sb[:, 0:4, :], wk4[:, 0:4, :])
        nc.sync.dma_start(wk_sb[:, 4:8, :], wk4[:, 4:8, :])
        nc.scalar.dma_start(wp_sb[:, 0:2, :], wp4[:, 0:2, :])
        nc.sync.dma_start(wp_sb[:, 2:4, :], wp4[:, 2:4, :])

        # 128x128 lower-block mask: keep (1.0) where i >= j, else 0.
        mask_tri = const.tile([128, 128], bf16)
        nc.gpsimd.memset(mask_tri[:], 1.0)
        nc.gpsimd.affine_select(
            out=mask_tri[:],
            in_=mask_tri[:],
            compare_op=mybir.AluOpType.is_ge,
            fill=0.0,
            base=0,
            pattern=[[1, 128]],
            channel_multiplier=-1,
        )

        # v for all heads, natural (t, d) layout, 65-wide blocks per head:
        # cols 0:64 = v, col 64 = ones (row-sum trick).  The 65-col
        # stationary loads are cheaper than 128-padded ones and need no
        # zero-fill memset.
        v_all = const.tile([128, MT * 8 * 65], bf16)
        v4 = v_all.rearrange("p (t h e) -> p t h e", t=MT, h=8)
        nc.gpsimd.memset(v4[:, :, :, 64:65], 1.0)

        # q^T for all 8 heads (bf16, 4KB/part each pair tile).
        qT_t = [const.tile([128, T], bf16, name=f"qT{p}") for p in range(PAIRS)]
        # k^T stored twice per pair with complementary zeroed halves so the
        # S matmul loads full 128-row weights (HAM sees a full array) while
        # streaming the fully-real shared qT pair tile.
        kT0_t = [const.tile([128, T], bf16, name=f"kT0{p}") for p in range(PAIRS)]
        kT1_t = [const.tile([128, T], bf16, name=f"kT1{p}") for p in range(PAIRS)]
        for p in range(PAIRS):
            nc.gpsimd.memset(kT0_t[p][64:128, :], 0.0)
            nc.vector.memset(kT1_t[p][0:64, :], 0.0)
        yT_tiles = [const.tile([128, T], bf16, name=f"yT{i}") for i in range(PAIRS)]

        # ---------------- Phase 0 (minimal): V t0-3 only --------------------
        # The entire QK0 projection defers into gated filler work inside
        # pair-0 attention (gates at row 0's S sections as the q/k quarters
        # are first consumed); V for t-tiles 4-15 defers likewise.  Inline
        # work is just v t0-3 (whose inputs arrive first) plus warm-keeper
        # matmuls covering the x/weight DMA tail, so attention starts at
        # the DMA floor (~20us) instead of after a serial projection phase.
        psA = [gpsum.tile([128, 512], f32, tag="gp", name=f"vA{t}")
               for t in range(4)]
        for k in range(KT):
            for t in range(4):
                nc.tensor.matmul(
                    psA[t][:],
                    lhsT=xsb[k][:, t * 128:(t + 1) * 128],
                    rhs=wv_sb[:, k, :],
                    start=(k == 0),
                    stop=(k == KT - 1),
                )
            warm_mm(2)
        for t in range(4):
            nc.vector.tensor_add(
                v4[:, t, :, 0:64],
                psA[t].rearrange("p (h e) -> p h e", h=8),
                bv_bc.rearrange("p (h e) -> p h e", h=8),
            )
        # pad PE activity across the remaining x/wq/wk DMA (~8us) so the
        # clock gate stays at 8/8 into the attention ramp.
        warm_mm(110)
        wps.release()
        gpsum.release()

        # ---------------- Attention with filler-slot pipelining -------------
        # One filler item is emitted into the PE queue after each S-section:
        # QK matmuls of pair p+1 during pair p<3, output-projection work
        # during pair 3 (gated on incremental normalization) and the tail.
        ptp = ctx.enter_context(tc.tile_pool(name="ptp", bufs=3))
        nrm = ctx.enter_context(tc.tile_pool(name="nrm", bufs=3))
        ostp = ctx.enter_context(tc.tile_pool(name="ost", bufs=3))
        # staged output-projection partials (split chunks 2-3): 16 live max
        ppart = ctx.enter_context(tc.tile_pool(name="ppart", bufs=16))
        sps = ctx.enter_context(tc.tile_pool(name="sps", bufs=2, space="PSUM"))
        yps = ctx.enter_context(tc.tile_pool(name="yps", bufs=4, space="PSUM"))
        qkp = ctx.enter_context(tc.tile_pool(name="qkp", bufs=2, space="PSUM"))

        fill_iters = deque()   # of (token, generator)
        done_toks = set()
        take_n = {"n": 1}

        def sprinkle():
            take = take_n["n"]
            while take > 0 and fill_iters:
                tok, g = fill_iters[0]
                th = next(g, None)
                if th is None:
                    done_toks.add(tok)
                    fill_iters.popleft()
                    continue
                th()
                take -= 1

        def sprinkle1():
            sv = take_n["n"]
            take_n["n"] = 1
            sprinkle()
            take_n["n"] = sv

        def flush(tok):
            while tok not in done_toks and fill_iters:
                t0, g = fill_iters[0]
                th = next(g, None)
                if th is None:
                    done_toks.add(t0)
                    fill_iters.popleft()
                    continue
                th()
            done_toks.add(tok)

        def drain():
            while fill_iters:
                sprinkle1()

        def g_qk0q(m):
            """Deferred pair-0 q projection quarter m."""
            ms = slice(m * 512, (m + 1) * 512)
            ps = qkp.tile([128, 512], f32, tag="qk", name=f"qk0q{m}")
            for k in range(KT):
                def mm(ps=ps, k=k, ms=ms):
                    nc.tensor.matmul(
                        ps[:], lhsT=wq_sb[:, k, 0:128], rhs=xsb[k][:, ms],
                        start=(k == 0), stop=(k == KT - 1))
                yield mm

            def ev(ps=ps, ms=ms):
                nc.vector.tensor_scalar_add(qT_t[0][:, ms], ps[:],
                                            bq_sb[:, 0:1])
            yield ev

        def g_qk0k(m):
            """Deferred pair-0 k projection quarter m."""
            ms = slice(m * 512, (m + 1) * 512)
            ps = qkp.tile([128, 512], f32, tag="qk", name=f"qk0k{m}")
            for k in range(KT):
                def mm(ps=ps, k=k, ms=ms):
                    nc.tensor.matmul(
                        ps[:], lhsT=wk_sb[:, k, 0:128], rhs=xsb[k][:, ms],
                        start=(k == 0), stop=(k == KT - 1))
                yield mm

            def ev(ps=ps, ms=ms):
                nc.vector.tensor_scalar_add(
                    kT0_t[0][0:64, ms], ps[0:64, :], bk_sb[0:64, 0:1])
                nc.vector.tensor_scalar_add(
                    kT1_t[0][64:128, ms], ps[64:128, :], bk_sb[64:128, 0:1])
            yield ev

        def g_v(group):
            """Deferred v for t-tiles 4g..4g+3 (t-outer, one qkp bank)."""
            for t in range(4 * group, 4 * group + 4):
                ps = qkp.tile([128, 512], f32, tag="qk", name=f"v{t}")
                for k in range(KT):
                    def mm(ps=ps, k=k, t=t):
                        nc.tensor.matmul(
                            ps[:],
                            lhsT=xsb[k][:, t * 128:(t + 1) * 128],
                            rhs=wv_sb[:, k, :],
                            start=(k == 0), stop=(k == KT - 1))
                    yield mm

                def ev(ps=ps, t=t):
                    nc.vector.tensor_add(
                        v4[:, t, :, 0:64],
                        ps.rearrange("p (h e) -> p h e", h=8),
                        bv_bc.rearrange("p (h e) -> p h e", h=8))
                yield ev

        def qk_gen(p):
            """Yield one-instruction thunks computing qT/kT for pair p."""
            for qk in range(2):
                w_sb = wq_sb if qk == 0 else wk_sb
                for m in range(4):
                    qk_ps = qkp.tile(
                        [128, 512], f32, tag="qk", name=f"qk{p}_{qk}_{m}"
                    )
                    for k in range(KT):
                        def mm(qk_ps=qk_ps, k=k, m=m, w_sb=w_sb):
                            nc.tensor.matmul(
                                qk_ps[:],
                                lhsT=w_sb[:, k, p * 128:(p + 1) * 128],
                                rhs=xsb[k][:, m * 512:(m + 1) * 512],
                                start=(k == 0),
                                stop=(k == KT - 1),
                            )
                        yield mm

                    def evict(qk_ps=qk_ps, qk=qk, m=m):
                        ms = slice(m * 512, (m + 1) * 512)
                        if qk == 0:
                            nc.vector.tensor_scalar_add(
                                qT_t[p][:, ms], qk_ps[:], bq_sb[:, p:p + 1]
                            )
                        else:
                            nc.vector.tensor_scalar_add(
                                kT0_t[p][0:64, ms], qk_ps[0:64, :],
                                bk_sb[0:64, p:p + 1],
                            )
                            nc.vector.tensor_scalar_add(
                                kT1_t[p][64:128, ms], qk_ps[64:128, :],
                                bk_sb[64:128, p:p + 1],
                            )
                    yield evict

        def proj_gen(ic):
            """Output projection for sequence tiles 4*ic..4*ic+3; valid once
            every head's chunk ic is normalized into yT_tiles."""
            for mt in range(4 * ic, 4 * ic + 4):
                ost = ostp.tile([128, C], bf16, tag="ost", name=f"ost{mt}")
                for oh in range(2):
                    pps = qkp.tile(
                        [128, 512], f32, tag="qk", name=f"pp{mt}_{oh}"
                    )
                    for k in range(4):
                        def mm(pps=pps, k=k, mt=mt, oh=oh):
                            nc.tensor.matmul(
                                pps[:],
                                lhsT=yT_tiles[k][:, mt * 128:(mt + 1) * 128],
                                rhs=wp_sb[:, k, oh * 512:(oh + 1) * 512],
                                start=(k == 0),
                                stop=(k == 3),
                            )
                        yield mm

                    def evict(pps=pps, ost=ost, mt=mt, oh=oh):
                        # DVE for one half, ACT for the other: in the drain
                        # tail exp is done and ACT is free, so the eviction
                        # latency never gates the qkp bank rotation.  DMA
                        # each half as soon as it lands so the writeback
                        # drains concurrently with the remaining matmuls.
                        if oh == 0:
                            nc.vector.tensor_copy(
                                ost[:, 0:512], pps[:]
                            )
                        else:
                            nc.scalar.activation(
                                ost[:, 512:1024], pps[:], AF.Copy
                            )
                        (nc.sync if (mt * 2 + oh) % 2 == 0
                         else nc.scalar).dma_start(
                            outD[mt * 128:(mt + 1) * 128,
                                 oh * 512:(oh + 1) * 512],
                            ost[:, oh * 512:(oh + 1) * 512],
                        )
                    yield evict

        # Chunks 2-3 of the output projection are split so pair-3's first
        # head gets real filler work: part A (pairs 0-2 plus pair-3 head 6)
        # stages a bf16 partial during head 6; part B adds head 7's
        # contribution and evicts.  This also shrinks the serial drain tail
        # to part B of chunk 3.
        part_sb = {}   # (mt, oh) -> staged partial tile

        def g_projA(ic):
            for mt in range(4 * ic, 4 * ic + 4):
                for oh in range(2):
                    pps = qkp.tile([128, 512], f32, tag="qk",
                                   name=f"pA{mt}_{oh}")
                    for k in range(4):
                        def mm(pps=pps, k=k, mt=mt, oh=oh):
                            if k < 3:
                                nc.tensor.matmul(
                                    pps[:],
                                    lhsT=yT_tiles[k][:, mt * 128:(mt + 1) * 128],
                                    rhs=wp_sb[:, k, oh * 512:(oh + 1) * 512],
                                    start=(k == 0),
                                    stop=False,
                                )
                            else:
                                nc.tensor.matmul(
                                    pps[:],
                                    lhsT=yT_tiles[3][0:64,
                                                     mt * 128:(mt + 1) * 128],
                                    rhs=wp_sb[0:64, 3, oh * 512:(oh + 1) * 512],
                                    start=False,
                                    stop=True,
                                )
                        yield mm

                    def evict(pps=pps, mt=mt, oh=oh):
                        pt = ppart.tile([128, 512], bf16, tag="pp",
                                        name=f"ps{mt}_{oh}")
                        part_sb[(mt, oh)] = pt
                        nc.vector.tensor_copy(pt[:], pps[:])
                    yield evict

        def g_projB(ic):
            for mt in range(4 * ic, 4 * ic + 4):
                ost = ostp.tile([128, C], bf16, tag="ost", name=f"ost{mt}")
                for oh in range(2):
                    pps = qkp.tile([128, 512], f32, tag="qk",
                                   name=f"pB{mt}_{oh}")

                    def mm(pps=pps, mt=mt, oh=oh):
                        nc.tensor.matmul(
                            pps[:],
                            lhsT=yT_tiles[3][64:128, mt * 128:(mt + 1) * 128],
                            rhs=wp_sb[64:128, 3, oh * 512:(oh + 1) * 512],
                            start=True,
                            stop=True,
                        )
                    yield mm

                    def evict(pps=pps, ost=ost, mt=mt, oh=oh):
                        nc.vector.tensor_add(
                            ost[:, oh * 512:(oh + 1) * 512],
                            part_sb[(mt, oh)][:], pps[:])
                        (nc.sync if (mt * 2 + oh) % 2 == 0
                         else nc.scalar).dma_start(
                            outD[mt * 128:(mt + 1) * 128,
                                 oh * 512:(oh + 1) * 512],
                            ost[:, oh * 512:(oh + 1) * 512],
                        )
                    yield evict

        # deferred phase-0 work, in first-consumed order (must match the
        # gate order: flush() drains the queue from the head)
        fill_iters.append(("k0", g_qk0k(0)))
        fill_iters.append(("q0", g_qk0q(0)))
        fill_iters.append(("q1", g_qk0q(1)))
        fill_iters.append(("q2", g_qk0q(2)))
        fill_iters.append(("q3", g_qk0q(3)))
        fill_iters.append(("k1", g_qk0k(1)))
        fill_iters.append(("v1", g_v(1)))
        fill_iters.append(("k2", g_qk0k(2)))
        fill_iters.append(("v2", g_v(2)))
        fill_iters.append(("k3", g_qk0k(3)))
        fill_iters.append(("v3", g_v(3)))

        if True:
            for p in range(PAIRS):
                if p < PAIRS - 1:
                    fill_iters.append((f"qk{p + 1}", qk_gen(p + 1)))
                # pairs 0/3 have far more filler supply than section points:
                # consume two thunks per section there.
                take_n["n"] = 2 if p in (0, 3) else 1
                qT = qT_t[p]
                for hh in range(2):
                    h = p * 2 + hh
                    kTt = kT0_t[p] if hh == 0 else kT1_t[p]
                    hs = slice(hh * 64, hh * 64 + 64)
                    ypt = [
                        yps.tile([128, 512], f32, tag="yps", name=f"y{p}_{hh}_{ic}")
                        for ic in range(4)
                    ]

                    def yt_chunks(jj):
                        out = []
                        for ic in range(jj // 4, 4):
                            a = max(ic * 512, 128 * jj)
                            out.append((ic, a, (ic + 1) * 512 - a))
                        return out

                    pending = None  # (jj, PT, chunks)
                    for j in range(MT + 1):
                        if p == 0 and j > 0 and j % 4 == 0 and j < MT:
                            # S row j consumes kT cols 512(j//4).. and the
                            # pending drain at row j+1 consumes v t-tile j:
                            # force any unconsumed deferred projections in.
                            flush(f"k{j // 4}")
                            flush(f"v{j // 4}")
                        if j < MT:
                            W = T - 128 * j
                            PT = ptp.tile(
                                [128, T], bf16, tag="pt", name=f"pt{p}_{hh}_{j}"
                            )
                            nsec = (W + 511) // 512
                            for s in range(nsec):
                                if p == 0 and j == 0:
                                    if s == 0:
                                        flush("k0")
                                        flush("q0")
                                    else:
                                        flush(f"q{s}")
                                sw = min(512, W - s * 512)
                                ps = sps.tile(
                                    [128, 512], f32, tag="sps",
                                    name=f"s{p}_{hh}_{j}_{s}"
                                )
                                io = 128 * j + s * 512
                                nc.tensor.matmul(
                                    ps[:, 0:sw],
                                    lhsT=kTt[:, j * 128:(j + 1) * 128],
                                    rhs=qT[:, io:io + sw],
                                    start=True,
                                    stop=True,
                                )
                                # interleave ~half the pending yT matmuls
                                # between S sections to keep the PE fed
                                if pending is not None and s == 0:
                                    jj, PTj, chunks = pending
                                    take = chunks[:max(1, len(chunks) // 2)]
                                    rest = chunks[len(take):]
                                    for ic, a, w2 in take:
                                        nc.tensor.matmul(
                                            ypt[ic][0:65, a - ic * 512:512],
                                            lhsT=v4[:, jj, h, 0:65],
                                            rhs=PTj[:, a - 128 * jj:
                                                    a - 128 * jj + w2],
                                            start=(jj == 0),
                                            stop=(jj == 4 * ic + 3),
                                        )
                                        if p in (0, 3):
                                            sprinkle1()
                                    pending = (jj, PTj, rest)
                                sprinkle()
                                nc.scalar.activation(
                                    PT[:, s * 512:s * 512 + sw],
                                    ps[:, 0:sw],
                                    AF.Exp,
                                    scale=SC,
                                )
                            # zero upper-triangular part of the diagonal block
                            nc.vector.tensor_mul(
                                PT[:, 0:128], PT[:, 0:128], mask_tri[:]
                            )
                        if pending is not None:
                            jj, PTj, chunks = pending
                            for ic, a, w2 in chunks:
                                nc.tensor.matmul(
                                    ypt[ic][0:65, a - ic * 512:512],
                                    lhsT=v4[:, jj, h, 0:65],
                                    rhs=PTj[:, a - 128 * jj:a - 128 * jj + w2],
                                    start=(jj == 0),
                                    stop=(jj == 4 * ic + 3),
                                )
                                if p in (0, 3):
                                    sprinkle1()
                        # chunk ic's accumulation closes with row 4*ic+3
                        # (drained above at j == 4*ic+4): normalize it now so
                        # ypt banks free early and, on the last head, the
                        # output projection for its sequence tiles can start.
                        if j > 0 and j % 4 == 0:
                            ic = j // 4 - 1
                            sums = nrm.tile([1, 512], f32, tag="sums",
                                            name=f"sm{p}_{hh}_{ic}")
                            nc.vector.tensor_copy(sums[:], ypt[ic][64:65, :])
                            rcp_row = nrm.tile([1, 512], f32, tag="rrow",
                                               name=f"rr{p}_{hh}_{ic}")
                            nc.vector.reciprocal_approx_fast(
                                rcp_row[:], sums[:]
                            )
                            rcp = nrm.tile([64, 512], f32, tag="rcp",
                                           name=f"rc{p}_{hh}_{ic}")
                            nc.gpsimd.partition_broadcast(rcp[:], rcp_row[:])
                            nc.vector.tensor_mul(
                                yT_tiles[p][hs, ic * 512:(ic + 1) * 512],
                                ypt[ic][0:64, :],
                                rcp[:],
                            )
                            if p == PAIRS - 1:
                                if hh == 0:
                                    if ic >= 2:
                                        fill_iters.append(
                                            (f"projA{ic}", g_projA(ic)))
                                elif ic < 2:
                                    fill_iters.append(
                                        (f"proj{ic}", proj_gen(ic)))
                                else:
                                    fill_iters.append(
                                        (f"projB{ic}", g_projB(ic)))
                        if j < MT:
                            pending = (j, PT, yt_chunks(j))
                # ensure pair p+1's qT/kT (and trailing proj work) are fully
                # emitted before the next pair's S sections enter the queue
                drain()

    nc.compile()
    return nc


def _get_nc():
    if "nc" not in _CACHE:
        _CACHE["nc"] = _build_nc()
    return _CACHE["nc"]


def make_in_maps(x, Wq, bq, Wk, bk, Wv, bv, Wp, bp):
    import ml_dtypes

    bf = ml_dtypes.bfloat16
    x = np.asarray(x, np.float32)
    Wq = np.asarray(Wq, np.float32).astype(bf)
    Wk = np.asarray(Wk, np.float32).astype(bf)
    Wv = np.asarray(Wv, np.float32).astype(bf)
    Wp = np.asarray(Wp, np.float32).astype(bf)
    bq = np.asarray(bq, np.float32)
    bk = np.asarray(bk, np.float32)
    bv = np.asarray(bv, np.float32)
    in_maps = []
    for c in range(NCORES):
        b, hg = divmod(c, 2)
        sl = slice(hg * 512, (hg + 1) * 512)
        in_maps.append({
            "xT": np.ascontiguousarray(x[b].T.astype(bf)),
            "wq": np.ascontiguousarray(Wq[:, sl]),
            "wk": np.ascontiguousarray(Wk[:, sl]),
            "wv": np.ascontiguousarray(Wv[:, sl]),
            "wp": np.ascontiguousarray(Wp[sl, :]),
            "bq": np.ascontiguousarray(bq[sl]),
            "bk": np.ascontiguousarray(bk[sl]),
            "bv": np.ascontiguousarray(bv[sl]),
        })
    return in_maps


def combine(results, bp):
    bp = np.asarray(bp, np.float32)
    out = np.empty((B, T, C), np.float32)
    for b in range(B):
        out[b] = (np.asarray(results[2 * b]["out"], np.float32)
                  + np.asarray(results[2 * b + 1]["out"], np.float32) + bp)
    return out


def kernel(x, Wq, bq, Wk, bk, Wv, bv, Wp, bp):
    from concourse import bass_utils

    nc = _get_nc()
    in_maps = make_in_maps(x, Wq, bq, Wk, bk, Wv, bv, Wp, bp)
    res = bass_utils.run_bass_kernel_spmd(nc, in_maps, core_ids=list(range(NCORES)))
    return combine(res.results, bp)



# revision 44
# speedup vs baseline: 1.0013x; 1.0013x over previous
"""Causal self-attention (B=4, T=2048, C=1024, H=16, D=64) on 8 trn2 NeuronCores.

Sharding: core c = (batch b=c//2, head-group hg=c%2 of 8 heads / 512 channels).
Each core computes attention for its 8 heads on its batch plus the partial
output projection over its 512 channels of Wp; the host sums the two partial
projections per batch and adds bp.

Per-core layout is feature-major ("transposed"): x is sent as xT (C, T) so
q/k project directly as qT = Wq.T @ x.T with both operands k(partition)-major.
v is computed in natural (T, D) orientation with a ones-column appended per
head so that the yT = [v|1].T @ P^T matmul also yields softmax row sums.
Matmul operands are bf16 (1 cyc/row on the PE); accumulation, softmax
internals and the final output stay fp32.

All attention matmuls are geometrically FULL 128x128-array ops (kT stored
twice per pair with complementary zero halves; v blocks padded to 128 wide)
so the PE HAM activity monitor keeps the clock gate at K=8/8 (2.4 GHz)
instead of dropping to 4/8 on the 64-row/65-col attention shapes.

Schedule: phase V (v for all heads, DMA-paced by sequence-half x loads) ->
QK(pair 0) m-outer -> per pair p: attention (software-pipelined per head:
yT matmuls of key-tile j-1 interleave between the S-matmul sections of
key-tile j), with independent full-array PE work streamed one item per
S-section into the PE's exp-wait gaps: the QK projection of pair p+1 for
p<3, and the output projection (gated on incremental per-chunk softmax
normalization) for p=3 and the tail.  The ACT engine runs only the softmax
exp during attention; all steady-state evictions go through the DVE.
"""

import math
from collections import deque

import numpy as np

B, T, C = 4, 2048, 1024
H, D = 16, 64
NCORES = 8
PAIRS = 4          # head pairs per core (2 heads = 128 channels each)
KT = C // 128      # 8 k-tiles over input channels
MT = T // 128      # 16 tiles over sequence
SC = 1.0 / math.sqrt(D)

_CACHE = {}


def _build_nc():
    from contextlib import ExitStack

    import concourse.bacc as bacc
    import concourse.mybir as mybir
    import concourse.tile as tile

    f32 = mybir.dt.float32
    bf16 = mybir.dt.bfloat16
    AF = mybir.ActivationFunctionType

    nc = bacc.Bacc("TRN2", target_bir_lowering=False, debug=False)

    xT = nc.dram_tensor("xT", (C, T), bf16, kind="ExternalInput").ap()
    wqD = nc.dram_tensor("wq", (C, 512), bf16, kind="ExternalInput").ap()
    wkD = nc.dram_tensor("wk", (C, 512), bf16, kind="ExternalInput").ap()
    wvD = nc.dram_tensor("wv", (C, 512), bf16, kind="ExternalInput").ap()
    wpD = nc.dram_tensor("wp", (512, C), bf16, kind="ExternalInput").ap()
    bqD = nc.dram_tensor("bq", (512,), f32, kind="ExternalInput").ap()
    bkD = nc.dram_tensor("bk", (512,), f32, kind="ExternalInput").ap()
    bvD = nc.dram_tensor("bv", (512,), f32, kind="ExternalInput").ap()
    # partial projections leave the core in bf16: halves the 8MB writeback
    # (it is ring-bandwidth-bound in the drain tail); the host sums the two
    # per-batch partials in fp32.
    outD = nc.dram_tensor("out", (T, C), bf16, kind="ExternalOutput").ap()

    with tile.TileContext(nc) as tc, ExitStack() as ctx:
        const = ctx.enter_context(tc.tile_pool(name="const", bufs=1))
        xp = ctx.enter_context(tc.tile_pool(name="xp", bufs=1))

        wv_sb = const.tile([128, KT, 512], bf16)
        xsb = [xp.tile([128, T], bf16, name=f"xsb{k}") for k in range(KT)]
        wq_sb = const.tile([128, KT, 512], bf16)
        wk_sb = const.tile([128, KT, 512], bf16)
        wp_sb = const.tile([128, 4, C], bf16)
        wz = const.tile([128, 512], bf16)     # warm-up zeros
        wdum = const.tile([128, 8], bf16)     # dummy exp target

        # ---- warm-up: the HAM clock gate defaults to 4/8 (1.2 GHz) and
        # un-throttles only after ~3.4us of sustained PE activity; dummy
        # matmuls from t~0 cover the initial DMA wait so phase 0 runs at
        # 2.4 GHz.  The dummy exp pulls the ACT table-set load (~2.7us)
        # out of the first attention row.
        gpsum = tc.alloc_tile_pool(name="gpsum", bufs=4, space="PSUM")
        wps = tc.alloc_tile_pool(name="wps", bufs=1, space="PSUM")
        wt = wps.tile([128, 512], f32, name="warm")
        nc.vector.memset(wz[:], 0.0)
        nc.scalar.activation(wdum[:], wz[:, 0:8], AF.Exp, scale=1.0)
        for _ in range(16):
            nc.tensor.matmul(wt[:], lhsT=wz[:, 0:128], rhs=wz[:], start=True,
                             stop=True)

        def warm_mm(n=1):
            # dummy self-loading matmuls (~50ns each issue-to-issue when
            # overlapped) to pad PE activity across known DMA waits; never
            # use standalone ldweights (walrus pairs it with the next real
            # matmul, which then consumes the dummy weights).
            for _ in range(n):
                nc.tensor.matmul(wt[:, 0:128], lhsT=wz[:, 0:128],
                                 rhs=wz[:, 0:128], start=True, stop=True)

        # DMA issue order = first-needed first.  Phase 0 only consumes
        # x cols 0:1024 (V t0-3, q m0/m1) plus wv/wq/wk; everything else
        # streams in behind attention's first rows.  Alternate big
        # transfers across both HWDGE rings (SP + ACT).
        bq_sb = const.tile([128, PAIRS], f32)
        nc.sync.dma_start(bq_sb[:], bqD.rearrange("(a p) -> p a", p=128))
        bk_sb = const.tile([128, PAIRS], f32)
        nc.scalar.dma_start(bk_sb[:], bkD.rearrange("(a p) -> p a", p=128))
        bv_row = const.tile([1, 512], f32)
        nc.sync.dma_start(bv_row[:], bvD.rearrange("(a n) -> a n", a=1))
        bv_bc = const.tile([128, 512], f32)
        nc.gpsimd.partition_broadcast(bv_bc[:], bv_row[:])
        wv4 = wvD.rearrange("(k p) n -> p k n", p=128)
        wq4 = wqD.rearrange("(k p) n -> p k n", p=128)
        wk4 = wkD.rearrange("(k p) n -> p k n", p=128)
        wp4 = wpD.rearrange("(k p) n -> p k n", p=128)
        for k in range(KT):
            exs = nc.sync if k % 2 == 0 else nc.scalar
            ewv = nc.scalar if k % 2 == 0 else nc.sync
            ewv.dma_start(wv_sb[:, k, :], wv4[:, k, :])
            exs.dma_start(xsb[k][:, 0:512], xT[k * 128:(k + 1) * 128, 0:512])
        nc.scalar.dma_start(wq_sb[:, 0:4, :], wq4[:, 0:4, :])
        nc.sync.dma_start(wq_sb[:, 4:8, :], wq4[:, 4:8, :])
        for k in range(KT):
            exs = nc.scalar if k % 2 == 0 else nc.sync
            exs.dma_start(xsb[k][:, 512:1024],
                          xT[k * 128:(k + 1) * 128, 512:1024])
        nc.scalar.dma_start(wk_sb[:, 0:4, :], wk4[:, 0:4, :])
        nc.sync.dma_start(wk_sb[:, 4:8, :], wk4[:, 4:8, :])
        for k in range(KT):
            exs = nc.sync if k % 2 == 0 else nc.scalar
            exs.dma_start(
                xsb[k][:, 1024:2048], xT[k * 128:(k + 1) * 128, 1024:2048]
            )
        nc.scalar.dma_start(wp_sb[:, 0:2, :], wp4[:, 0:2, :])
        nc.sync.dma_start(wp_sb[:, 2:4, :], wp4[:, 2:4, :])

        # 128x128 lower-block mask: keep (1.0) where i >= j, else 0.
        mask_tri = const.tile([128, 128], bf16)
        nc.gpsimd.memset(mask_tri[:], 1.0)
        nc.gpsimd.affine_select(
            out=mask_tri[:],
            in_=mask_tri[:],
            compare_op=mybir.AluOpType.is_ge,
            fill=0.0,
            base=0,
            pattern=[[1, 128]],
            channel_multiplier=-1,
        )

        # v for all heads, natural (t, d) layout, 65-wide blocks per head:
        # cols 0:64 = v, col 64 = ones (row-sum trick).  The 65-col
        # stationary loads are cheaper than 128-padded ones and need no
        # zero-fill memset.
        v_all = const.tile([128, MT * 8 * 65], bf16)
        v4 = v_all.rearrange("p (t h e) -> p t h e", t=MT, h=8)
        nc.gpsimd.memset(v4[:, :, :, 64:65], 1.0)

        # q^T for all 8 heads (bf16, 4KB/part each pair tile).
        qT_t = [const.tile([128, T], bf16, name=f"qT{p}") for p in range(PAIRS)]
        # k^T stored twice per pair with complementary zeroed halves so the
        # S matmul loads full 128-row weights (HAM sees a full array) while
        # streaming the fully-real shared qT pair tile.
        kT0_t = [const.tile([128, T], bf16, name=f"kT0{p}") for p in range(PAIRS)]
        kT1_t = [const.tile([128, T], bf16, name=f"kT1{p}") for p in range(PAIRS)]
        for p in range(PAIRS):
            nc.gpsimd.memset(kT0_t[p][64:128, :], 0.0)
            nc.vector.memset(kT1_t[p][0:64, :], 0.0)
        yT_tiles = [const.tile([128, T], bf16, name=f"yT{i}") for i in range(PAIRS)]

        # ---------------- Phase 0, paced to the DMA stream ------------------
        # Inbound DMA only flows from ~11us to ~38us (~300 GB/s after a
        # ~10us runtime ramp), so phase 0 is ordered to consume each tensor
        # as it lands: V t0-3 (wv + x cols 0:512), q m0 (wq), q m1 + V t4-7
        # (x cols 512:1024), k m0/m1 (wk), q m2/m3 (x cols 1024:2048).
        # The remainder (k m2/m3, v t8-15, ~17us) defers into gated filler
        # work inside pair-0 attention, which has ~10us of genuine PE slack
        # under its exp schedule.
        def v_block(ts):
            ps = [gpsum.tile([128, 512], f32, tag="gp", name=f"v{t}")
                  for t in ts]
            for k in range(KT):
                for i, t in enumerate(ts):
                    nc.tensor.matmul(
                        ps[i][:],
                        lhsT=xsb[k][:, t * 128:(t + 1) * 128],
                        rhs=wv_sb[:, k, :],
                        start=(k == 0),
                        stop=(k == KT - 1),
                    )
                warm_mm(2)
            for i, t in enumerate(ts):
                nc.vector.tensor_add(
                    v4[:, t, :, 0:64],
                    ps[i].rearrange("p (h e) -> p h e", h=8),
                    bv_bc.rearrange("p (h e) -> p h e", h=8),
                )

        def qk0_group(qk, m):
            ms = slice(m * 512, (m + 1) * 512)
            w_sb = wq_sb if qk == 0 else wk_sb
            ps = gpsum.tile([128, 512], f32, tag="gp", name=f"qk0_{qk}_{m}")
            for k in range(KT):
                nc.tensor.matmul(
                    ps[:],
                    lhsT=w_sb[:, k, 0:128],
                    rhs=xsb[k][:, ms],
                    start=(k == 0),
                    stop=(k == KT - 1),
                )
            if qk == 0:
                nc.vector.tensor_scalar_add(
                    qT_t[0][:, ms], ps[:], bq_sb[:, 0:1]
                )
            else:
                # ACT is idle during this phase; use it for k evictions
                nc.scalar.activation(
                    kT0_t[0][0:64, ms], ps[0:64, :],
                    AF.Identity, bias=bk_sb[0:64, 0:1],
                )
                nc.scalar.activation(
                    kT1_t[0][64:128, ms], ps[64:128, :],
                    AF.Identity, bias=bk_sb[64:128, 0:1],
                )

        v_block([0, 1, 2, 3])
        qk0_group(0, 0)
        warm_mm(4)
        qk0_group(0, 1)
        v_block([4, 5, 6, 7])
        qk0_group(1, 0)
        warm_mm(4)
        qk0_group(1, 1)
        warm_mm(4)
        qk0_group(0, 2)
        warm_mm(4)
        qk0_group(0, 3)
        warm_mm(12)
        wps.release()
        gpsum.release()

        # ---------------- Attention with filler-slot pipelining -------------
        # One filler item is emitted into the PE queue after each S-section:
        # QK matmuls of pair p+1 during pair p<3, output-projection work
        # during pair 3 (gated on incremental normalization) and the tail.
        ptp = ctx.enter_context(tc.tile_pool(name="ptp", bufs=3))
        nrm = ctx.enter_context(tc.tile_pool(name="nrm", bufs=3))
        ostp = ctx.enter_context(tc.tile_pool(name="ost", bufs=3))
        # staged output-projection partials (split chunks 2-3): 16 live max
        ppart = ctx.enter_context(tc.tile_pool(name="ppart", bufs=16))
        sps = ctx.enter_context(tc.tile_pool(name="sps", bufs=2, space="PSUM"))
        yps = ctx.enter_context(tc.tile_pool(name="yps", bufs=4, space="PSUM"))
        qkp = ctx.enter_context(tc.tile_pool(name="qkp", bufs=2, space="PSUM"))

        fill_iters = deque()   # of (token, generator)
        done_toks = set()
        take_n = {"n": 1}

        def sprinkle():
            take = take_n["n"]
            while take > 0 and fill_iters:
                tok, g = fill_iters[0]
                th = next(g, None)
                if th is None:
                    done_toks.add(tok)
                    fill_iters.popleft()
                    continue
                th()
                take -= 1

        def sprinkle1():
            sv = take_n["n"]
            take_n["n"] = 1
            sprinkle()
            take_n["n"] = sv

        def flush(tok):
            while tok not in done_toks and fill_iters:
                t0, g = fill_iters[0]
                th = next(g, None)
                if th is None:
                    done_toks.add(t0)
                    fill_iters.popleft()
                    continue
                th()
            done_toks.add(tok)

        def drain():
            while fill_iters:
                sprinkle1()

        def g_qk0q(m):
            """Deferred pair-0 q projection quarter m."""
            ms = slice(m * 512, (m + 1) * 512)
            ps = qkp.tile([128, 512], f32, tag="qk", name=f"qk0q{m}")
            for k in range(KT):
                def mm(ps=ps, k=k, ms=ms):
                    nc.tensor.matmul(
                        ps[:], lhsT=wq_sb[:, k, 0:128], rhs=xsb[k][:, ms],
                        start=(k == 0), stop=(k == KT - 1))
                yield mm

            def ev(ps=ps, ms=ms):
                nc.vector.tensor_scalar_add(qT_t[0][:, ms], ps[:],
                                            bq_sb[:, 0:1])
            yield ev

        def g_qk0k(m):
            """Deferred pair-0 k projection quarter m."""
            ms = slice(m * 512, (m + 1) * 512)
            ps = qkp.tile([128, 512], f32, tag="qk", name=f"qk0k{m}")
            for k in range(KT):
                def mm(ps=ps, k=k, ms=ms):
                    nc.tensor.matmul(
                        ps[:], lhsT=wk_sb[:, k, 0:128], rhs=xsb[k][:, ms],
                        start=(k == 0), stop=(k == KT - 1))
                yield mm

            def ev(ps=ps, ms=ms):
                nc.vector.tensor_scalar_add(
                    kT0_t[0][0:64, ms], ps[0:64, :], bk_sb[0:64, 0:1])
                nc.vector.tensor_scalar_add(
                    kT1_t[0][64:128, ms], ps[64:128, :], bk_sb[64:128, 0:1])
            yield ev

        def g_v(group):
            """Deferred v for t-tiles 4g..4g+3 (t-outer, one qkp bank)."""
            for t in range(4 * group, 4 * group + 4):
                ps = qkp.tile([128, 512], f32, tag="qk", name=f"v{t}")
                for k in range(KT):
                    def mm(ps=ps, k=k, t=t):
                        nc.tensor.matmul(
                            ps[:],
                            lhsT=xsb[k][:, t * 128:(t + 1) * 128],
                            rhs=wv_sb[:, k, :],
                            start=(k == 0), stop=(k == KT - 1))
                    yield mm

                def ev(ps=ps, t=t):
                    nc.vector.tensor_add(
                        v4[:, t, :, 0:64],
                        ps.rearrange("p (h e) -> p h e", h=8),
                        bv_bc.rearrange("p (h e) -> p h e", h=8))
                yield ev

        def qk_gen(p):
            """Yield one-instruction thunks computing qT/kT for pair p."""
            for qk in range(2):
                w_sb = wq_sb if qk == 0 else wk_sb
                for m in range(4):
                    qk_ps = qkp.tile(
                        [128, 512], f32, tag="qk", name=f"qk{p}_{qk}_{m}"
                    )
                    for k in range(KT):
                        def mm(qk_ps=qk_ps, k=k, m=m, w_sb=w_sb):
                            nc.tensor.matmul(
                                qk_ps[:],
                                lhsT=w_sb[:, k, p * 128:(p + 1) * 128],
                                rhs=xsb[k][:, m * 512:(m + 1) * 512],
                                start=(k == 0),
                                stop=(k == KT - 1),
                            )
                        yield mm

                    def evict(qk_ps=qk_ps, qk=qk, m=m):
                        ms = slice(m * 512, (m + 1) * 512)
                        if qk == 0:
                            nc.vector.tensor_scalar_add(
                                qT_t[p][:, ms], qk_ps[:], bq_sb[:, p:p + 1]
                            )
                        else:
                            nc.vector.tensor_scalar_add(
                                kT0_t[p][0:64, ms], qk_ps[0:64, :],
                                bk_sb[0:64, p:p + 1],
                            )
                            nc.vector.tensor_scalar_add(
                                kT1_t[p][64:128, ms], qk_ps[64:128, :],
                                bk_sb[64:128, p:p + 1],
                            )
                    yield evict

        def proj_gen(ic):
            """Output projection for sequence tiles 4*ic..4*ic+3; valid once
            every head's chunk ic is normalized into yT_tiles."""
            for mt in range(4 * ic, 4 * ic + 4):
                ost = ostp.tile([128, C], bf16, tag="ost", name=f"ost{mt}")
                for oh in range(2):
                    pps = qkp.tile(
                        [128, 512], f32, tag="qk", name=f"pp{mt}_{oh}"
                    )
                    for k in range(4):
                        def mm(pps=pps, k=k, mt=mt, oh=oh):
                            nc.tensor.matmul(
                                pps[:],
                                lhsT=yT_tiles[k][:, mt * 128:(mt + 1) * 128],
                                rhs=wp_sb[:, k, oh * 512:(oh + 1) * 512],
                                start=(k == 0),
                                stop=(k == 3),
                            )
                        yield mm

                    def evict(pps=pps, ost=ost, mt=mt, oh=oh):
                        # DVE for one half, ACT for the other: in the drain
                        # tail exp is done and ACT is free, so the eviction
                        # latency never gates the qkp bank rotation.  DMA
                        # each half as soon as it lands so the writeback
                        # drains concurrently with the remaining matmuls.
                        if oh == 0:
                            nc.vector.tensor_copy(
                                ost[:, 0:512], pps[:]
                            )
                        else:
                            nc.scalar.activation(
                                ost[:, 512:1024], pps[:], AF.Copy
                            )
                        (nc.sync if (mt * 2 + oh) % 2 == 0
                         else nc.scalar).dma_start(
                            outD[mt * 128:(mt + 1) * 128,
                                 oh * 512:(oh + 1) * 512],
                            ost[:, oh * 512:(oh + 1) * 512],
                        )
                    yield evict

        # Chunks 2-3 of the output projection are split so pair-3's first
        # head gets real filler work: part A (pairs 0-2 plus pair-3 head 6)
        # stages a bf16 partial during head 6; part B adds head 7's
        # contribution and evicts.  This also shrinks the serial drain tail
        # to part B of chunk 3.
        part_sb = {}   # (mt, oh) -> staged partial tile

        def g_projA(ic):
            for mt in range(4 * ic, 4 * ic + 4):
                for oh in range(2):
                    pps = qkp.tile([128, 512], f32, tag="qk",
                                   name=f"pA{mt}_{oh}")
                    for k in range(4):
                        def mm(pps=pps, k=k, mt=mt, oh=oh):
                            if k < 3:
                                nc.tensor.matmul(
                                    pps[:],
                                    lhsT=yT_tiles[k][:, mt * 128:(mt + 1) * 128],
                                    rhs=wp_sb[:, k, oh * 512:(oh + 1) * 512],
                                    start=(k == 0),
                                    stop=False,
                                )
                            else:
                                nc.tensor.matmul(
                                    pps[:],
                                    lhsT=yT_tiles[3][0:64,
                                                     mt * 128:(mt + 1) * 128],
                                    rhs=wp_sb[0:64, 3, oh * 512:(oh + 1) * 512],
                                    start=False,
                                    stop=True,
                                )
                        yield mm

                    def evict(pps=pps, mt=mt, oh=oh):
                        pt = ppart.tile([128, 512], bf16, tag="pp",
                                        name=f"ps{mt}_{oh}")
                        part_sb[(mt, oh)] = pt
                        nc.vector.tensor_copy(pt[:], pps[:])
                    yield evict

        def g_projB(ic):
            for mt in range(4 * ic, 4 * ic + 4):
                ost = ostp.tile([128, C], bf16, tag="ost", name=f"ost{mt}")
                for oh in range(2):
                    pps = qkp.tile([128, 512], f32, tag="qk",
                                   name=f"pB{mt}_{oh}")

                    def mm(pps=pps, mt=mt, oh=oh):
                        nc.tensor.matmul(
                            pps[:],
                            lhsT=yT_tiles[3][64:128, mt * 128:(mt + 1) * 128],
                            rhs=wp_sb[64:128, 3, oh * 512:(oh + 1) * 512],
                            start=True,
                            stop=True,
                        )
                    yield mm

                    def evict(pps=pps, ost=ost, mt=mt, oh=oh):
                        nc.vector.tensor_add(
                            ost[:, oh * 512:(oh + 1) * 512],
                            part_sb[(mt, oh)][:], pps[:])
                        (nc.sync if (mt * 2 + oh) % 2 == 0
                         else nc.scalar).dma_start(
                            outD[mt * 128:(mt + 1) * 128,
                                 oh * 512:(oh + 1) * 512],
                            ost[:, oh * 512:(oh + 1) * 512],
                        )
                    yield evict

        # deferred phase-0 work, in first-consumed order (must match the
        # gate order: flush() drains the queue from the head)
        fill_iters.append(("k2", g_qk0k(2)))
        fill_iters.append(("v2", g_v(2)))
        fill_iters.append(("k3", g_qk0k(3)))
        fill_iters.append(("v3", g_v(3)))

        if True:
            for p in range(PAIRS):
                if p < PAIRS - 1:
                    fill_iters.append((f"qk{p + 1}", qk_gen(p + 1)))
                # pair 0 has more filler supply than section points: consume
                # two thunks per section there; elsewhere stay at one per
                # section so filler matmuls never delay the exp chain.
                take_n["n"] = 2 if p == 0 else 1
                qT = qT_t[p]
                for hh in range(2):
                    h = p * 2 + hh
                    kTt = kT0_t[p] if hh == 0 else kT1_t[p]
                    hs = slice(hh * 64, hh * 64 + 64)
                    ypt = [
                        yps.tile([128, 512], f32, tag="yps", name=f"y{p}_{hh}_{ic}")
                        for ic in range(4)
                    ]

                    def yt_chunks(jj):
                        out = []
                        for ic in range(jj // 4, 4):
                            a = max(ic * 512, 128 * jj)
                            out.append((ic, a, (ic + 1) * 512 - a))
                        return out

                    pending = None  # (jj, PT, chunks)
                    for j in range(MT + 1):
                        if p == 0 and j > 0 and j % 4 == 0 and j < MT:
                            # S row j consumes kT cols 512(j//4).. and the
                            # pending drain at row j+1 consumes v t-tile j:
                            # force any unconsumed deferred projections in.
                            flush(f"k{j // 4}")
                            flush(f"v{j // 4}")
                        if j < MT:
                            W = T - 128 * j
                            PT = ptp.tile(
                                [128, T], bf16, tag="pt", name=f"pt{p}_{hh}_{j}"
                            )
                            nsec = (W + 511) // 512
                            for s in range(nsec):
                                sw = min(512, W - s * 512)
                                ps = sps.tile(
                                    [128, 512], f32, tag="sps",
                                    name=f"s{p}_{hh}_{j}_{s}"
                                )
                                io = 128 * j + s * 512
                                nc.tensor.matmul(
                                    ps[:, 0:sw],
                                    lhsT=kTt[:, j * 128:(j + 1) * 128],
                                    rhs=qT[:, io:io + sw],
                                    start=True,
                                    stop=True,
                                )
                                # interleave ~half the pending yT matmuls
                                # between S sections to keep the PE fed
                                if pending is not None and s == 0:
                                    jj, PTj, chunks = pending
                                    take = chunks[:max(1, len(chunks) // 2)]
                                    rest = chunks[len(take):]
                                    for ic, a, w2 in take:
                                        nc.tensor.matmul(
                                            ypt[ic][0:65, a - ic * 512:512],
                                            lhsT=v4[:, jj, h, 0:65],
                                            rhs=PTj[:, a - 128 * jj:
                                                    a - 128 * jj + w2],
                                            start=(jj == 0),
                                            stop=(jj == 4 * ic + 3),
                                        )
                                        if p == 0:
                                            sprinkle1()
                                    pending = (jj, PTj, rest)
                                sprinkle()
                                nc.scalar.activation(
                                    PT[:, s * 512:s * 512 + sw],
                                    ps[:, 0:sw],
                                    AF.Exp,
                                    scale=SC,
                                )
                            # zero upper-triangular part of the diagonal block
                            nc.vector.tensor_mul(
                                PT[:, 0:128], PT[:, 0:128], mask_tri[:]
                            )
                        if pending is not None:
                            jj, PTj, chunks = pending
                            for ic, a, w2 in chunks:
                                nc.tensor.matmul(
                                    ypt[ic][0:65, a - ic * 512:512],
                                    lhsT=v4[:, jj, h, 0:65],
                                    rhs=PTj[:, a - 128 * jj:a - 128 * jj + w2],
                                    start=(jj == 0),
                                    stop=(jj == 4 * ic + 3),
                                )
                                if p == 0:
                                    sprinkle1()
                        # chunk ic's accumulation closes with row 4*ic+3
                        # (drained above at j == 4*ic+4): normalize it now so
                        # ypt banks free early and, on the last head, the
                        # output projection for its sequence tiles can start.
                        if j > 0 and j % 4 == 0:
                            ic = j // 4 - 1
                            sums = nrm.tile([1, 512], f32, tag="sums",
                                            name=f"sm{p}_{hh}_{ic}")
                            nc.vector.tensor_copy(sums[:], ypt[ic][64:65, :])
                            rcp_row = nrm.tile([1, 512], f32, tag="rrow",
                                               name=f"rr{p}_{hh}_{ic}")
                            nc.vector.reciprocal_approx_fast(
                                rcp_row[:], sums[:]
                            )
                            rcp = nrm.tile([64, 512], f32, tag="rcp",
                                           name=f"rc{p}_{hh}_{ic}")
                            nc.gpsimd.partition_broadcast(rcp[:], rcp_row[:])
                            nc.vector.tensor_mul(
                                yT_tiles[p][hs, ic * 512:(ic + 1) * 512],
                                ypt[ic][0:64, :],
                                rcp[:],
                            )
                            if p == PAIRS - 1:
                                if hh == 0:
                                    if ic >= 2:
                                        fill_iters.append(
                                            (f"projA{ic}", g_projA(ic)))
                                elif ic < 2:
                                    fill_iters.append(
                                        (f"proj{ic}", proj_gen(ic)))
                                else:
                                    fill_iters.append(
                                        (f"projB{ic}", g_projB(ic)))
                        if j < MT:
                            pending = (j, PT, yt_chunks(j))
                # ensure pair p+1's qT/kT (and trailing proj work) are fully
                # emitted before the next pair's S sections enter the queue
                drain()

    nc.compile()
    return nc


def _get_nc():
    if "nc" not in _CACHE:
        _CACHE["nc"] = _build_nc()
    return _CACHE["nc"]


def make_in_maps(x, Wq, bq, Wk, bk, Wv, bv, Wp, bp):
    import ml_dtypes

    bf = ml_dtypes.bfloat16
    x = np.asarray(x, np.float32)
    Wq = np.asarray(Wq, np.float32).astype(bf)
    Wk = np.asarray(Wk, np.float32).astype(bf)
    Wv = np.asarray(Wv, np.float32).astype(bf)
    Wp = np.asarray(Wp, np.float32).astype(bf)
    bq = np.asarray(bq, np.float32)
    bk = np.asarray(bk, np.float32)
    bv = np.asarray(bv, np.float32)
    in_maps = []
    for c in range(NCORES):
        b, hg = divmod(c, 2)
        sl = slice(hg * 512, (hg + 1) * 512)
        in_maps.append({
            "xT": np.ascontiguousarray(x[b].T.astype(bf)),
            "wq": np.ascontiguousarray(Wq[:, sl]),
            "wk": np.ascontiguousarray(Wk[:, sl]),
            "wv": np.ascontiguousarray(Wv[:, sl]),
            "wp": np.ascontiguousarray(Wp[sl, :]),
            "bq": np.ascontiguousarray(bq[sl]),
            "bk": np.ascontiguousarray(bk[sl]),
            "bv": np.ascontiguousarray(bv[sl]),
        })
    return in_maps


def combine(results, bp):
    bp = np.asarray(bp, np.float32)
    out = np.empty((B, T, C), np.float32)
    for b in range(B):
        out[b] = (np.asarray(results[2 * b]["out"], np.float32)
                  + np.asarray(results[2 * b + 1]["out"], np.float32) + bp)
    return out


def kernel(x, Wq, bq, Wk, bk, Wv, bv, Wp, bp):
    from concourse import bass_utils

    nc = _get_nc()
    in_maps = make_in_maps(x, Wq, bq, Wk, bk, Wv, bv, Wp, bp)
    res = bass_utils.run_bass_kernel_spmd(nc, in_maps, core_ids=list(range(NCORES)))
    return combine(res.results, bp)



# revision 45
# speedup vs baseline: 1.0511x; 1.0497x over previous
"""Causal self-attention (B=4, T=2048, C=1024, H=16, D=64) on 8 trn2 NeuronCores.

Sharding: core c = (batch b=c//2, head-group hg=c%2 of 8 heads / 512 channels).
Each core computes attention for its 8 heads on its batch plus the partial
output projection over its 512 channels of Wp; the host sums the two partial
projections per batch and adds bp.

Per-core layout is feature-major ("transposed"): x is sent as xT (C, T) so
q/k project directly as qT = Wq.T @ x.T with both operands k(partition)-major.
v is computed in natural (T, D) orientation with a ones-column appended per
head so that the yT = [v|1].T @ P^T matmul also yields softmax row sums.
Matmul operands are bf16 (1 cyc/row on the PE); accumulation, softmax
internals and the final output stay fp32.

All attention matmuls are geometrically FULL 128x128-array ops (kT stored
twice per pair with complementary zero halves; v blocks padded to 128 wide)
so the PE HAM activity monitor keeps the clock gate at K=8/8 (2.4 GHz)
instead of dropping to 4/8 on the 64-row/65-col attention shapes.

Schedule: phase V (v for all heads, DMA-paced by sequence-half x loads) ->
QK(pair 0) m-outer -> per pair p: attention (software-pipelined per head:
yT matmuls of key-tile j-1 interleave between the S-matmul sections of
key-tile j), with independent full-array PE work streamed one item per
S-section into the PE's exp-wait gaps: the QK projection of pair p+1 for
p<3, and the output projection (gated on incremental per-chunk softmax
normalization) for p=3 and the tail.  The ACT engine runs only the softmax
exp during attention; all steady-state evictions go through the DVE.
"""

import math
from collections import deque

import numpy as np

B, T, C = 4, 2048, 1024
H, D = 16, 64
NCORES = 8
PAIRS = 4          # head pairs per core (2 heads = 128 channels each)
KT = C // 128      # 8 k-tiles over input channels
MT = T // 128      # 16 tiles over sequence
SC = 1.0 / math.sqrt(D)

_CACHE = {}


def _build_nc():
    from contextlib import ExitStack

    import concourse.bacc as bacc
    import concourse.mybir as mybir
    import concourse.tile as tile

    f32 = mybir.dt.float32
    bf16 = mybir.dt.bfloat16
    AF = mybir.ActivationFunctionType

    nc = bacc.Bacc("TRN2", target_bir_lowering=False, debug=False)

    xT = nc.dram_tensor("xT", (C, T), bf16, kind="ExternalInput").ap()
    wqD = nc.dram_tensor("wq", (C, 512), bf16, kind="ExternalInput").ap()
    wkD = nc.dram_tensor("wk", (C, 512), bf16, kind="ExternalInput").ap()
    wvD = nc.dram_tensor("wv", (C, 512), bf16, kind="ExternalInput").ap()
    wpD = nc.dram_tensor("wp", (512, C), bf16, kind="ExternalInput").ap()
    bqD = nc.dram_tensor("bq", (512,), f32, kind="ExternalInput").ap()
    bkD = nc.dram_tensor("bk", (512,), f32, kind="ExternalInput").ap()
    bvD = nc.dram_tensor("bv", (512,), f32, kind="ExternalInput").ap()
    # partial projections leave the core in bf16: halves the 8MB writeback
    # (it is ring-bandwidth-bound in the drain tail); the host sums the two
    # per-batch partials in fp32.
    outD = nc.dram_tensor("out", (T, C), bf16, kind="ExternalOutput").ap()

    with tile.TileContext(nc) as tc, ExitStack() as ctx:
        const = ctx.enter_context(tc.tile_pool(name="const", bufs=1))
        xp = ctx.enter_context(tc.tile_pool(name="xp", bufs=1))

        wv_sb = const.tile([128, KT, 512], bf16)
        xsb = [xp.tile([128, T], bf16, name=f"xsb{k}") for k in range(KT)]
        wq_sb = const.tile([128, KT, 512], bf16)
        wk_sb = const.tile([128, KT, 512], bf16)
        wp_sb = const.tile([128, 4, C], bf16)

        # DMA issue order = first-needed first.  V-phase t-group 0 needs only
        # (wv slice k, xsb[k] cols 0:1024); the second sequence halves and the
        # remaining weights stream in behind while the PE is already busy.
        # Alternate big transfers across both HWDGE rings (SP + ACT).
        bq_sb = const.tile([128, PAIRS], f32)
        nc.sync.dma_start(bq_sb[:], bqD.rearrange("(a p) -> p a", p=128))
        bk_sb = const.tile([128, PAIRS], f32)
        nc.scalar.dma_start(bk_sb[:], bkD.rearrange("(a p) -> p a", p=128))
        bv_row = const.tile([1, 512], f32)
        nc.sync.dma_start(bv_row[:], bvD.rearrange("(a n) -> a n", a=1))
        bv_bc = const.tile([128, 512], f32)
        nc.gpsimd.partition_broadcast(bv_bc[:], bv_row[:])
        wv4 = wvD.rearrange("(k p) n -> p k n", p=128)
        for k in range(KT):
            exs = nc.sync if k % 2 == 0 else nc.scalar
            ewv = nc.scalar if k % 2 == 0 else nc.sync
            ewv.dma_start(wv_sb[:, k, :], wv4[:, k, :])
            exs.dma_start(xsb[k][:, 0:1024], xT[k * 128:(k + 1) * 128, 0:1024])
        wq4 = wqD.rearrange("(k p) n -> p k n", p=128)
        nc.scalar.dma_start(wq_sb[:, 0:4, :], wq4[:, 0:4, :])
        nc.sync.dma_start(wq_sb[:, 4:8, :], wq4[:, 4:8, :])
        for k in range(KT):
            exs = nc.scalar if k % 2 == 0 else nc.sync
            exs.dma_start(
                xsb[k][:, 1024:2048], xT[k * 128:(k + 1) * 128, 1024:2048]
            )
        wk4 = wkD.rearrange("(k p) n -> p k n", p=128)
        nc.scalar.dma_start(wk_sb[:, 0:4, :], wk4[:, 0:4, :])
        nc.sync.dma_start(wk_sb[:, 4:8, :], wk4[:, 4:8, :])
        wp4 = wpD.rearrange("(k p) n -> p k n", p=128)
        nc.scalar.dma_start(wp_sb[:, 0:2, :], wp4[:, 0:2, :])
        nc.sync.dma_start(wp_sb[:, 2:4, :], wp4[:, 2:4, :])

        # 128x128 lower-block mask: keep (1.0) where i >= j, else 0.
        mask_tri = const.tile([128, 128], bf16)
        nc.gpsimd.memset(mask_tri[:], 1.0)
        nc.gpsimd.affine_select(
            out=mask_tri[:],
            in_=mask_tri[:],
            compare_op=mybir.AluOpType.is_ge,
            fill=0.0,
            base=0,
            pattern=[[1, 128]],
            channel_multiplier=-1,
        )

        # v for all heads, natural (t, d) layout, 128-wide blocks per head:
        # cols 0:64 = v, col 64 = ones (row-sum trick), cols 65:128 = zeros.
        # Full-width weight loads keep the PE HAM activity monitor at K=8/8.
        v_all = const.tile([128, MT * 8 * 128], bf16)
        nc.vector.memset(v_all[:], 0.0)
        v4 = v_all.rearrange("p (t h e) -> p t h e", t=MT, h=8)
        nc.gpsimd.memset(v4[:, :, :, 64:65], 1.0)

        # q^T for all 8 heads (bf16, 4KB/part each pair tile).
        qT_t = [const.tile([128, T], bf16, name=f"qT{p}") for p in range(PAIRS)]
        # k^T stored twice per pair with complementary zeroed halves so the
        # S matmul loads full 128-row weights (HAM sees a full array) while
        # streaming the fully-real shared qT pair tile.
        kT0_t = [const.tile([128, T], bf16, name=f"kT0{p}") for p in range(PAIRS)]
        kT1_t = [const.tile([128, T], bf16, name=f"kT1{p}") for p in range(PAIRS)]
        for p in range(PAIRS):
            nc.gpsimd.memset(kT0_t[p][64:128, :], 0.0)
            nc.vector.memset(kT1_t[p][0:64, :], 0.0)
        yT_tiles = [const.tile([128, T], bf16, name=f"yT{i}") for i in range(PAIRS)]

        # ---------------- Phase V + QK(0), DMA-aware interleave -------------
        # V t-group 0 and QK0's m=0,1 tiles touch only the first sequence
        # halves of x, so they run while the second halves stream in; V
        # t-group 1 and QK0 m=2,3 follow.  All share one 8-slot PSUM pool.
        gpsum = tc.alloc_tile_pool(name="gpsum", bufs=8, space="PSUM")

        def v_group(tg):
            ps = [
                gpsum.tile([128, 512], f32, tag="gp", name=f"vps{tg}_{t}")
                for t in range(8)
            ]
            for k in range(KT):
                for t8 in range(8):
                    tt = tg * 8 + t8
                    nc.tensor.matmul(
                        ps[t8][:],
                        lhsT=xsb[k][:, tt * 128:(tt + 1) * 128],
                        rhs=wv_sb[:, k, :],
                        start=(k == 0),
                        stop=(k == KT - 1),
                    )
            for t8 in range(8):
                tt = tg * 8 + t8
                nc.vector.tensor_add(
                    v4[:, tt, :, 0:64],
                    ps[t8].rearrange("p (h e) -> p h e", h=8),
                    bv_bc.rearrange("p (h e) -> p h e", h=8),
                )

        def qk0_group(qk, m):
            ms = slice(m * 512, (m + 1) * 512)
            w_sb = wq_sb if qk == 0 else wk_sb
            ps = gpsum.tile([128, 512], f32, tag="gp", name=f"qk0_{qk}_{m}")
            for k in range(KT):
                nc.tensor.matmul(
                    ps[:],
                    lhsT=w_sb[:, k, 0:128],
                    rhs=xsb[k][:, ms],
                    start=(k == 0),
                    stop=(k == KT - 1),
                )
            if qk == 0:
                nc.vector.tensor_scalar_add(
                    qT_t[0][:, ms], ps[:], bq_sb[:, 0:1]
                )
            else:
                # ACT is idle during this phase; use it for k evictions
                nc.scalar.activation(
                    kT0_t[0][0:64, ms], ps[0:64, :],
                    AF.Identity, bias=bk_sb[0:64, 0:1],
                )
                nc.scalar.activation(
                    kT1_t[0][64:128, ms], ps[64:128, :],
                    AF.Identity, bias=bk_sb[64:128, 0:1],
                )

        # Ordered by DMA arrival: wq lands before the x second halves, which
        # land before wk — so all q-projections run between the two V groups
        # and the k-projections close the phase.  No block starts before its
        # inputs arrive, so the PE never idles into a HAM re-throttle.
        v_group(0)
        for m in range(4):
            qk0_group(0, m)
        v_group(1)
        for m in range(4):
            qk0_group(1, m)
        gpsum.release()

        # ---------------- Attention with filler-slot pipelining -------------
        # One filler item is emitted into the PE queue after each S-section:
        # QK matmuls of pair p+1 during pair p<3, output-projection work
        # during pair 3 (gated on incremental normalization) and the tail.
        ptp = ctx.enter_context(tc.tile_pool(name="ptp", bufs=3))
        nrm = ctx.enter_context(tc.tile_pool(name="nrm", bufs=3))
        ostp = ctx.enter_context(tc.tile_pool(name="ost", bufs=3))
        sps = ctx.enter_context(tc.tile_pool(name="sps", bufs=2, space="PSUM"))
        yps = ctx.enter_context(tc.tile_pool(name="yps", bufs=4, space="PSUM"))
        qkp = ctx.enter_context(tc.tile_pool(name="qkp", bufs=2, space="PSUM"))

        fill_iters = deque()

        def sprinkle():
            while fill_iters:
                th = next(fill_iters[0], None)
                if th is None:
                    fill_iters.popleft()
                    continue
                th()
                return

        def drain():
            while fill_iters:
                sprinkle()

        def qk_gen(p):
            """Yield one-instruction thunks computing qT/kT for pair p."""
            for qk in range(2):
                w_sb = wq_sb if qk == 0 else wk_sb
                for m in range(4):
                    qk_ps = qkp.tile(
                        [128, 512], f32, tag="qk", name=f"qk{p}_{qk}_{m}"
                    )
                    for k in range(KT):
                        def mm(qk_ps=qk_ps, k=k, m=m, w_sb=w_sb):
                            nc.tensor.matmul(
                                qk_ps[:],
                                lhsT=w_sb[:, k, p * 128:(p + 1) * 128],
                                rhs=xsb[k][:, m * 512:(m + 1) * 512],
                                start=(k == 0),
                                stop=(k == KT - 1),
                            )
                        yield mm

                    def evict(qk_ps=qk_ps, qk=qk, m=m):
                        ms = slice(m * 512, (m + 1) * 512)
                        if qk == 0:
                            nc.vector.tensor_scalar_add(
                                qT_t[p][:, ms], qk_ps[:], bq_sb[:, p:p + 1]
                            )
                        else:
                            nc.vector.tensor_scalar_add(
                                kT0_t[p][0:64, ms], qk_ps[0:64, :],
                                bk_sb[0:64, p:p + 1],
                            )
                            nc.vector.tensor_scalar_add(
                                kT1_t[p][64:128, ms], qk_ps[64:128, :],
                                bk_sb[64:128, p:p + 1],
                            )
                    yield evict

        def proj_gen(ic):
            """Output projection for sequence tiles 4*ic..4*ic+3; valid once
            every head's chunk ic is normalized into yT_tiles."""
            for mt in range(4 * ic, 4 * ic + 4):
                ost = ostp.tile([128, C], bf16, tag="ost", name=f"ost{mt}")
                for oh in range(2):
                    pps = qkp.tile(
                        [128, 512], f32, tag="qk", name=f"pp{mt}_{oh}"
                    )
                    for k in range(4):
                        def mm(pps=pps, k=k, mt=mt, oh=oh):
                            nc.tensor.matmul(
                                pps[:],
                                lhsT=yT_tiles[k][:, mt * 128:(mt + 1) * 128],
                                rhs=wp_sb[:, k, oh * 512:(oh + 1) * 512],
                                start=(k == 0),
                                stop=(k == 3),
                            )
                        yield mm

                    def evict(pps=pps, ost=ost, mt=mt, oh=oh):
                        # DVE for one half, ACT for the other: in the drain
                        # tail exp is done and ACT is free, so the eviction
                        # latency never gates the qkp bank rotation.  DMA
                        # each half as soon as it lands so the writeback
                        # drains concurrently with the remaining matmuls.
                        if oh == 0:
                            nc.vector.tensor_copy(
                                ost[:, 0:512], pps[:]
                            )
                        else:
                            nc.scalar.activation(
                                ost[:, 512:1024], pps[:], AF.Copy
                            )
                        (nc.sync if (mt * 2 + oh) % 2 == 0
                         else nc.scalar).dma_start(
                            outD[mt * 128:(mt + 1) * 128,
                                 oh * 512:(oh + 1) * 512],
                            ost[:, oh * 512:(oh + 1) * 512],
                        )
                    yield evict

        if True:
            for p in range(PAIRS):
                if p < PAIRS - 1:
                    fill_iters.append(qk_gen(p + 1))
                qT = qT_t[p]
                for hh in range(2):
                    h = p * 2 + hh
                    kTt = kT0_t[p] if hh == 0 else kT1_t[p]
                    hs = slice(hh * 64, hh * 64 + 64)
                    ypt = [
                        yps.tile([128, 512], f32, tag="yps", name=f"y{p}_{hh}_{ic}")
                        for ic in range(4)
                    ]

                    def yt_chunks(jj):
                        out = []
                        for ic in range(jj // 4, 4):
                            a = max(ic * 512, 128 * jj)
                            out.append((ic, a, (ic + 1) * 512 - a))
                        return out

                    pending = None  # (jj, PT, chunks)
                    for j in range(MT + 1):
                        if j < MT:
                            W = T - 128 * j
                            PT = ptp.tile(
                                [128, T], bf16, tag="pt", name=f"pt{p}_{hh}_{j}"
                            )
                            nsec = (W + 511) // 512
                            for s in range(nsec):
                                sw = min(512, W - s * 512)
                                ps = sps.tile(
                                    [128, 512], f32, tag="sps",
                                    name=f"s{p}_{hh}_{j}_{s}"
                                )
                                io = 128 * j + s * 512
                                nc.tensor.matmul(
                                    ps[:, 0:sw],
                                    lhsT=kTt[:, j * 128:(j + 1) * 128],
                                    rhs=qT[:, io:io + sw],
                                    start=True,
                                    stop=True,
                                )
                                # interleave ~half the pending yT matmuls
                                # between S sections to keep the PE fed
                                if pending is not None and s == 0:
                                    jj, PTj, chunks = pending
                                    take = chunks[:max(1, len(chunks) // 2)]
                                    rest = chunks[len(take):]
                                    for ic, a, w2 in take:
                                        nc.tensor.matmul(
                                            ypt[ic][:, a - ic * 512:512],
                                            lhsT=v4[:, jj, h, :],
                                            rhs=PTj[:, a - 128 * jj:
                                                    a - 128 * jj + w2],
                                            start=(jj == 0),
                                            stop=(jj == 4 * ic + 3),
                                        )
                                    pending = (jj, PTj, rest)
                                sprinkle()
                                nc.scalar.activation(
                                    PT[:, s * 512:s * 512 + sw],
                                    ps[:, 0:sw],
                                    AF.Exp,
                                    scale=SC,
                                )
                            # zero upper-triangular part of the diagonal block
                            nc.vector.tensor_mul(
                                PT[:, 0:128], PT[:, 0:128], mask_tri[:]
                            )
                        if pending is not None:
                            jj, PTj, chunks = pending
                            for ic, a, w2 in chunks:
                                nc.tensor.matmul(
                                    ypt[ic][:, a - ic * 512:512],
                                    lhsT=v4[:, jj, h, :],
                                    rhs=PTj[:, a - 128 * jj:a - 128 * jj + w2],
                                    start=(jj == 0),
                                    stop=(jj == 4 * ic + 3),
                                )
                        # chunk ic's accumulation closes with row 4*ic+3
                        # (drained above at j == 4*ic+4): normalize it now so
                        # ypt banks free early and, on the last head, the
                        # output projection for its sequence tiles can start.
                        if j > 0 and j % 4 == 0:
                            ic = j // 4 - 1
                            sums = nrm.tile([1, 512], f32, tag="sums",
                                            name=f"sm{p}_{hh}_{ic}")
                            nc.vector.tensor_copy(sums[:], ypt[ic][64:65, :])
                            rcp_row = nrm.tile([1, 512], f32, tag="rrow",
                                               name=f"rr{p}_{hh}_{ic}")
                            nc.vector.reciprocal_approx_fast(
                                rcp_row[:], sums[:]
                            )
                            rcp = nrm.tile([64, 512], f32, tag="rcp",
                                           name=f"rc{p}_{hh}_{ic}")
                            nc.gpsimd.partition_broadcast(rcp[:], rcp_row[:])
                            nc.vector.tensor_mul(
                                yT_tiles[p][hs, ic * 512:(ic + 1) * 512],
                                ypt[ic][0:64, :],
                                rcp[:],
                            )
                            if p == PAIRS - 1 and hh == 1:
                                fill_iters.append(proj_gen(ic))
                        if j < MT:
                            pending = (j, PT, yt_chunks(j))
                # ensure pair p+1's qT/kT (and trailing proj work) are fully
                # emitted before the next pair's S sections enter the queue
                drain()

    nc.compile()
    return nc


def _get_nc():
    if "nc" not in _CACHE:
        _CACHE["nc"] = _build_nc()
    return _CACHE["nc"]


def make_in_maps(x, Wq, bq, Wk, bk, Wv, bv, Wp, bp):
    import ml_dtypes

    bf = ml_dtypes.bfloat16
    x = np.asarray(x, np.float32)
    Wq = np.asarray(Wq, np.float32).astype(bf)
    Wk = np.asarray(Wk, np.float32).astype(bf)
    Wv = np.asarray(Wv, np.float32).astype(bf)
    Wp = np.asarray(Wp, np.float32).astype(bf)
    bq = np.asarray(bq, np.float32)
    bk = np.asarray(bk, np.float32)
    bv = np.asarray(bv, np.float32)
    in_maps = []
    for c in range(NCORES):
        b, hg = divmod(c, 2)
        sl = slice(hg * 512, (hg + 1) * 512)
        in_maps.append({
            "xT": np.ascontiguousarray(x[b].T.astype(bf)),
            "wq": np.ascontiguousarray(Wq[:, sl]),
            "wk": np.ascontiguousarray(Wk[:, sl]),
            "wv": np.ascontiguousarray(Wv[:, sl]),
            "wp": np.ascontiguousarray(Wp[sl, :]),
            "bq": np.ascontiguousarray(bq[sl]),
            "bk": np.ascontiguousarray(bk[sl]),
            "bv": np.ascontiguousarray(bv[sl]),
        })
    return in_maps


def combine(results, bp):
    bp = np.asarray(bp, np.float32)
    out = np.empty((B, T, C), np.float32)
    for b in range(B):
        out[b] = (np.asarray(results[2 * b]["out"], np.float32)
                  + np.asarray(results[2 * b + 1]["out"], np.float32) + bp)
    return out


def kernel(x, Wq, bq, Wk, bk, Wv, bv, Wp, bp):
    from concourse import bass_utils

    nc = _get_nc()
    in_maps = make_in_maps(x, Wq, bq, Wk, bk, Wv, bv, Wp, bp)
    res = bass_utils.run_bass_kernel_spmd(nc, in_maps, core_ids=list(range(NCORES)))
    return combine(res.results, bp)

